# revision 1
# baseline (speedup 1.0000x reference)
"""Trainium2 Bass kernel for nn_DSA_11192684773671 (retrieval_knn).

Sharding: 8 cores = 4 batch items x 2 row-halves. Odd cores work on a
row-flipped view of their batch item (with ky-flipped conv weights), so the
single SPMD program is identical across cores; per-core differences live in
the input data only. Per frame, each core computes attention reads for its
half with one-sided row halos (no mid-frame communication), runs the two
fused conv blocks, and the pair exchanges fusion halves with one 2-core
AllGather.

Matmuls run as float32r (TF32-like, ~1.5e-4 rel rounding) at full PE rate.
Convs are 9 shifted matmuls over an x-padded [rows,34] layout with 36-element
zero pre/post-ambles (no per-shift range truncation needed).
"""
import math
import numpy as np

import concourse.bacc as bacc
import concourse.bass as bass
import concourse.mybir as mybir
import concourse.tile as tile
from concourse.bass_utils import run_bass_kernel_spmd
from concourse.masks import make_identity

F32 = mybir.dt.float32
F32R = mybir.dt.float32r
AF = mybir.ActivationFunctionType
ALU = mybir.AluOpType
AX = mybir.AxisListType

B, T, C, H, W = 4, 5, 512, 32, 32
HW = H * W
SQC = math.sqrt(C)
N_CORES = 8
PAIRS = [[0, 1], [2, 3], [4, 5], [6, 7]]

R_LR, R_SF, R_RLG, R_LF, R_RIG, R_OUT = 20, 18, 19, 18, 17, 16
QL, QS = R_LR * W, R_SF * W          # 640, 576
# q tiles: (q0, nq)
QT_L = [(0, 128), (128, 128), (256, 128), (384, 128), (512, 128)]
QT_S = [(0, 128), (128, 128), (256, 128), (384, 128), (512, 64)]
# qh groups: (col0, width, qt indices)
QH_L = [(0, 384, [0, 1, 2]), (384, 256, [3, 4])]
QH_S = [(0, 256, [0, 1]), (256, 320, [2, 3, 4])]


def PL(r):
    return 34 * r + 72  # 36 zero preamble + data + 36 zero postamble


def _sr_lr(idx):
    sr0, sr2 = max(idx - 1, 0), min(idx + 1, T - 1)
    lr = [i for i in range(T) if i not in (sr0, idx, sr2)]
    return sr0, sr2, lr


def _col_splits(r_out):
    L = 34 * r_out
    L1 = (L // 2 + 1) & ~1
    return [(0, L1), (L1, L)]


def build(frames=T):
    nc = bacc.Bacc(num_devices=N_CORES)

    feat0 = nc.declare_dram_parameter("feat0", [T, C, HW], F32, isOutput=False)
    featT0 = nc.declare_dram_parameter("featT0", [T, HW, C], F32, isOutput=False)
    asq0 = nc.declare_dram_parameter("asq0", [T, HW], F32, isOutput=False)
    w_lg1 = nc.declare_dram_parameter("w_lg1", [9, 1024, 512], F32, isOutput=False)
    w_lg2 = nc.declare_dram_parameter("w_lg2", [9, 512, 512], F32, isOutput=False)
    w_lgd = nc.declare_dram_parameter("w_lgd", [9, 1024, 512], F32, isOutput=False)
    w_ig1 = nc.declare_dram_parameter("w_ig1", [9, 1536, 512], F32, isOutput=False)
    w_ig2 = nc.declare_dram_parameter("w_ig2", [9, 512, 512], F32, isOutput=False)
    w_igd = nc.declare_dram_parameter("w_igd", [9, 1536, 512], F32, isOutput=False)
    b_lg1 = nc.declare_dram_parameter("b_lg1", [512], F32, isOutput=False)
    b_lgf = nc.declare_dram_parameter("b_lgf", [512], F32, isOutput=False)
    b_ig1 = nc.declare_dram_parameter("b_ig1", [512], F32, isOutput=False)
    b_igf = nc.declare_dram_parameter("b_igf", [512], F32, isOutput=False)
    selw = nc.declare_dram_parameter("selw", [2], F32, isOutput=False)
    fw = [nc.declare_dram_parameter(f"feat_out{t}", [C, HW], F32,
                                    isOutput=True) for t in range(T)]
    fTw = [nc.dram_tensor(f"fTw{t}", [HW, C], F32) for t in range(T)]
    asqw = [nc.dram_tensor(f"asqw{t}", [HW], F32) for t in range(T)]
    cc_in = [nc.dram_tensor(f"cc_in{t}", [C, 512], F32) for t in range(frames)]
    cc_out = [nc.dram_tensor(f"cc_out{t}", [2, C, 512], F32) for t in range(frames)]

    with tile.TileContext(nc) as tc:
        with (
            tc.tile_pool(name="persist", bufs=1) as pp,
            tc.tile_pool(name="wblk", bufs=8) as wpool,
            tc.tile_pool(name="frame", bufs=4) as fp,
        ):
            # constants
            ident = pp.tile([128, 128], F32)
            make_identity(nc, ident[:])
            one_col = pp.tile([128, 1], F32)
            nc.vector.memset(one_col[:], 1.0)
            ones2 = pp.tile([128, 2], F32R)
            nc.vector.memset(ones2[:].bitcast(F32), 1.0)
            bias_sb = {}
            for nm, hdl in (("lg1", b_lg1), ("lgf", b_lgf), ("ig1", b_ig1),
                            ("igf", b_igf)):
                t = pp.tile([128, 4], F32, tag=f"bias_{nm}", name=f"bias_{nm}")
                hap = hdl[:]
                src = bass.AP(tensor=hap.tensor, offset=0,
                              ap=[[1, 128], [128, 4]])
                nc.sync.dma_start(out=t[:], in_=src)
                bias_sb[nm] = t
            w0_sb = pp.tile([128, 1], F32)
            w1_sb = pp.tile([128, 1], F32)
            sap = selw[:]
            nc.sync.dma_start(out=w0_sb[:], in_=bass.AP(
                tensor=sap.tensor, offset=0, ap=[[0, 128], [1, 1]]))
            nc.sync.dma_start(out=w1_sb[:], in_=bass.AP(
                tensor=sap.tensor, offset=1, ap=[[0, 128], [1, 1]]))

            # frame sources: working copies once updated, inputs before
            def f_src(f, idx):
                return fw[f][:] if f < idx else feat0[f]

            def fT_src(f, idx):
                return fTw[f][:] if f < idx else featT0[f]

            def asq_src(f, idx):
                return asqw[f][:] if f < idx else asq0[f]

            def zero_pads(t, r):
                f = t[:].bitcast(F32)
                nc.vector.memset(f[:, 0:36], 0.0)
                nc.vector.memset(f[:, 36 + 34 * r:], 0.0)
                v3 = f[:, 36:36 + 34 * r].rearrange("p (r c) -> p r c", c=34)
                nc.vector.memset(v3[:, :, 0:1], 0.0)
                nc.vector.memset(v3[:, :, 33:34], 0.0)

            def data3(t, r0, nr):
                """[128, nr, 32] view of rows [r0, r0+nr) data region."""
                f = t[:]
                core = f[:, 36 + 34 * r0: 36 + 34 * (r0 + nr)]
                return core.rearrange("p (r c) -> p r c", c=34)[:, :, 1:33]

            def emit_logits(lp, aff_tiles, qts, kt_list, anchor, kpool, apool,
                            idx):
                """aff[qt][:nq, kcol:kcol+512] = 2/sqrt(C)*anchor^T K - asq/sqrt(C)."""
                for ki, (f, off) in enumerate(kt_list):
                    asqb = apool.tile([128, 512], F32, tag="asqb", name="asqb")
                    aap = asq_src(f, idx)
                    nc.scalar.dma_start(out=asqb[:], in_=bass.AP(
                        tensor=aap.tensor, offset=aap.offset + off,
                        ap=[[0, 128], [1, 512]]))
                    kbs = []
                    fsrc = f_src(f, idx)
                    for c in range(4):
                        kb = kpool.tile([128, 512], F32R, tag="kblk", name="kblk")
                        nc.sync.dma_start(
                            out=kb[:],
                            in_=fsrc[c * 128:(c + 1) * 128,
                                     off:off + 512].bitcast(F32R))
                        kbs.append(kb)
                    for (q0, nq), aff in zip(qts, aff_tiles):
                        ps = lp.tile([128, 512], F32, tag="lg", name="lg")
                        for c in range(4):
                            nc.tensor.matmul(
                                ps[0:nq, :], anchor[c][:, q0:q0 + nq], kbs[c][:],
                                start=(c == 0), stop=(c == 3))
                        nc.vector.tensor_tensor(
                            out=aff[0:nq, ki * 512:(ki + 1) * 512],
                            in0=ps[0:nq, :], in1=asqb[0:nq, :], op=ALU.add)

            def emit_softmax(aff, nq, K, sp, normalize):
                nm = sp.tile([128, 1], F32, tag="nm", name="nm")
                nc.vector.reduce_max(out=nm[0:nq], in_=aff[0:nq, 0:K],
                                     axis=AX.X, negate=True)
                den = sp.tile([128, 1], F32, tag="den", name="den")
                nc.scalar.activation(aff[0:nq, 0:K], aff[0:nq, 0:K], AF.Exp,
                                     bias=nm[0:nq], scale=1.0,
                                     accum_out=den[0:nq])
                rec = sp.tile([128, 1], F32, tag="rec", name="rec")
                nc.vector.reciprocal(rec[0:nq], den[0:nq])
                if normalize:
                    nc.vector.tensor_scalar(
                        out=aff[0:nq, 0:K], in0=aff[0:nq, 0:K],
                        scalar1=rec[0:nq], scalar2=None, op0=ALU.mult)
                return rec

            def emit_reads(tp_ps, rd_ps, rT_pool, ktpool, aff_tiles, qts, group,
                           col0, width, kc_list, n_kc, evac, idx):
                """resT = transpose(aff cols), read[ct] = sum_kc fT^T @ resT."""
                ps_ct = [rd_ps.tile([128, 384], F32, tag="rd", name="rd")
                         for _ in range(4)]
                for kci, (f, o) in enumerate(kc_list):
                    rt = rT_pool.tile([128, 384], F32R, tag="rT", name="rT")
                    base = 0
                    for qi in group:
                        q0, nq = qts[qi]
                        pst = tp_ps.tile([128, 128], F32, tag="tp", name="tp")
                        nc.tensor.transpose(
                            pst[:], aff_tiles[qi][:, kci * 128:(kci + 1) * 128],
                            ident[:])
                        nc.vector.tensor_copy(rt[:, base:base + nq],
                                               pst[:, 0:nq])
                        base += nq
                    kt = ktpool.tile([128, 512], F32R, tag="ktb", name="ktb")
                    nc.gpsimd.dma_start(
                        out=kt[:], in_=fT_src(f, idx)[o:o + 128, :]
                        .bitcast(F32R))
                    for ct in range(4):
                        nc.tensor.matmul(
                            ps_ct[ct][:, 0:width],
                            kt[:, ct * 128:(ct + 1) * 128],
                            rt[:, 0:width],
                            start=(kci == 0), stop=(kci == n_kc - 1))
                for ct in range(4):
                    evac(ct, ps_ct[ct], col0, width)

            def emit_convgroup(parts, r_out, cpool, evac):
                cols = _col_splits(r_out)
                total = sum(9 * n for (_, _, n) in parts)
                psums = {}
                cnt = 0
                for (wd, xts, n_icc) in parts:
                    for s in range(9):
                        dy, dx = s // 3, s % 3
                        delta = (dy - 1) * 34 + (dx - 1)
                        for icc in range(n_icc):
                            wb = wpool.tile([128, 512], F32R, tag="wblk", name="wblk")
                            # weights stream from the (otherwise idle) Pool
                            # engine so their dispatch doesn't serialize with
                            # the key/featT streams on SP
                            nc.gpsimd.dma_start(
                                out=wb[:],
                                in_=wd[s, icc * 128:(icc + 1) * 128, :]
                                .bitcast(F32R))
                            cnt += 1
                            for oc in range(4):
                                for (a, b) in cols:
                                    key = (oc, a)
                                    if key not in psums:
                                        psums[key] = cpool.tile(
                                            [128, 512], F32, tag="cv", name="cv")
                                    nc.tensor.matmul(
                                        psums[key][:, 0:b - a],
                                        wb[:, oc * 128:(oc + 1) * 128],
                                        xts[icc][:, 36 + a + delta:
                                                 36 + b + delta],
                                        start=(cnt == 1), stop=(cnt == total))
                for oc in range(4):
                    for (a, b) in cols:
                        evac(oc, a, b, psums[(oc, a)])

            # ---------------- frame loop ----------------
            for idx in range(frames):
                sr0, sr2, lr = _sr_lr(idx)
                Tl = len(lr)

                cur = [fp.tile([128, PL(R_LR)], F32R, tag="cur", name="cur")
                       for _ in range(4)]
                for c in range(4):
                    zero_pads(cur[c], R_LR)
                    nc.scalar.dma_start(
                        out=data3(cur[c], 0, R_LR),
                        in_=feat0[idx][c * 128:(c + 1) * 128, 0:QL]
                        .rearrange("p (r c) -> p r c", c=32).bitcast(F32R))
                sf = [fp.tile([128, PL(R_SF)], F32R, tag="sf", name="sf")
                      for _ in range(4)]
                lrp = [fp.tile([128, PL(R_LR)], F32R, tag="lrp", name="lrp")
                       for _ in range(4)]
                for c in range(4):
                    zero_pads(sf[c], R_SF)
                    zero_pads(lrp[c], R_LR)

                with (
                    tc.tile_pool(name="attn", bufs=4) as ap_,
                    tc.tile_pool(name="selfn", bufs=5) as snp,
                    tc.tile_pool(name="rT", bufs=6) as rTp,
                    tc.tile_pool(name="kstr", bufs=10) as kpool,
                    tc.tile_pool(name="sm", bufs=8) as sp_,
                ):
                    # anchor, scaled 2/sqrt(C), fp32r
                    anchor = [ap_.tile([128, QL], F32R, tag="anc", name="anc")
                              for _ in range(4)]
                    for c in range(4):
                        nc.scalar.activation(
                            anchor[c][:].rearrange("p (r c) -> p r c", c=32),
                            data3(cur[c], 0, R_LR),
                            AF.Copy, bias=0.0, scale=2.0 / SQC)

                    # ---- self affinity (normalized), all 5 q-tiles ----
                    self_n = [snp.tile([128, HW], F32, tag="sn", name="sn")
                              for _ in range(5)]
                    with tc.tile_pool(name="lps", bufs=4,
                                      space="PSUM") as lp:
                        emit_logits(lp, self_n, QT_L, [(idx, 0), (idx, 512)],
                                    anchor, kpool, ap_, idx)
                    for (q0, nq), sn in zip(QT_L, self_n):
                        emit_softmax(sn[:], nq, HW, sp_, True)

                    # ---- short range ----
                    kt_s = [(sr0, 0), (sr0, 512), (sr2, 0), (sr2, 512)]
                    kc_s = [(f, o) for f in (sr0, sr2)
                            for o in range(0, HW, 128)]

                    def sf_evac(ct, ps, col0, width):
                        r0, nr = col0 // 32, width // 32
                        nc.vector.scalar_tensor_tensor(
                            out=data3(sf[ct], r0, nr),
                            in0=ps[:, 0:width].rearrange(
                                "p (r c) -> p r c", c=32),
                            scalar=one_col[:], op0=ALU.mult, op1=ALU.add,
                            in1=data3(cur[ct], r0, nr))

                    # short logits for all 5 q-tiles in one key pass
                    with tc.tile_pool(name="affs", bufs=5) as affsp:
                        affs_t = [affsp.tile([128, 2 * HW], F32, tag="as",
                                             name="as") for _ in range(5)]
                        with tc.tile_pool(name="lps", bufs=4,
                                          space="PSUM") as lp:
                            emit_logits(lp, affs_t, QT_S, kt_s, anchor,
                                        kpool, ap_, idx)
                        for (q0, nq), a in zip(QT_S, affs_t):
                            emit_softmax(a[:], nq, 2 * HW, sp_, True)
                        with (
                            tc.tile_pool(name="tps", bufs=2,
                                         space="PSUM") as tp_ps,
                            tc.tile_pool(name="rps", bufs=4,
                                         space="PSUM") as rd_ps,
                        ):
                            for (col0, width, group) in QH_S:
                                emit_reads(tp_ps, rd_ps, rTp, kpool,
                                           affs_t, QT_S, group, col0, width,
                                           kc_s, len(kc_s), sf_evac, idx)

                    # ---- long range ----
                    kt_l = [(f, o) for f in lr for o in (0, 512)]
                    kc_l = [(f, o) for f in lr for o in range(0, HW, 128)]

                    def lr_evac(ct, ps, col0, width):
                        r0, nr = col0 // 32, width // 32
                        nc.vector.tensor_copy(
                            data3(lrp[ct], r0, nr),
                            ps[:, 0:width].rearrange("p (r c) -> p r c", c=32))

                    with tc.tile_pool(name="affl", bufs=3) as afflp:
                        for (col0, width, group) in QH_L:
                            affs = [afflp.tile([128, 3072], F32, tag="aff",
                                               name="aff") for _ in group]
                            afft = [None] * 5
                            for gi, qi in zip(range(len(group)), group):
                                afft[qi] = affs[gi]
                            with tc.tile_pool(name="lps", bufs=4,
                                              space="PSUM") as lp:
                                emit_logits(lp, [affs[i] for i in
                                                 range(len(group))],
                                            [QT_L[i] for i in group], kt_l,
                                            anchor, kpool, ap_, idx)
                            for gi, qi in zip(range(len(group)), group):
                                q0, nq = QT_L[qi]
                                rec = emit_softmax(affs[gi][:], nq, Tl * HW,
                                                   sp_, False)
                                for ti in range(Tl):
                                    nc.vector.scalar_tensor_tensor(
                                        out=affs[gi][0:nq,
                                                     ti * HW:(ti + 1) * HW],
                                        in0=affs[gi][0:nq,
                                                     ti * HW:(ti + 1) * HW],
                                        scalar=rec[0:nq], op0=ALU.mult,
                                        op1=ALU.subtract,
                                        in1=self_n[qi][0:nq, :])
                                nc.scalar.activation(
                                    affs[gi][0:nq, 0:Tl * HW],
                                    affs[gi][0:nq, 0:Tl * HW],
                                    AF.Abs, bias=0.0, scale=1.0)
                            with (
                                tc.tile_pool(name="tps", bufs=2,
                                             space="PSUM") as tp_ps,
                                tc.tile_pool(name="rps", bufs=4,
                                             space="PSUM") as rd_ps,
                            ):
                                emit_reads(tp_ps, rd_ps, rTp, kpool,
                                           afft, QT_L, group, col0, width,
                                           kc_l, len(kc_l), lr_evac, idx)

                # ---- convs ----
                fus = [None] * 4
                with (
                    tc.tile_pool(name="cvsb", bufs=4) as cvp,
                    tc.tile_pool(name="cvps", bufs=8, space="PSUM") as cps,
                ):
                    rcur = [cvp.tile([128, PL(R_LR)], F32R, tag="rcur", name="rcur")
                            for _ in range(4)]
                    rlr = [cvp.tile([128, PL(R_LR)], F32R, tag="rlr", name="rlr")
                           for _ in range(4)]
                    for c in range(4):
                        nc.scalar.activation(rcur[c][:], cur[c][:], AF.Relu,
                                             bias=0.0, scale=1.0)
                        nc.scalar.activation(rlr[c][:], lrp[c][:], AF.Relu,
                                             bias=0.0, scale=1.0)

                    # lg fuse
                    r1lg = [cvp.tile([128, PL(R_RLG)], F32R, tag="r1lg", name="r1lg")
                            for _ in range(4)]
                    lf = [cvp.tile([128, PL(R_SF)], F32R, tag="lf", name="lf")
                          for _ in range(4)]

                    def r1lg_evac(oc, a, b, ps):
                        nc.scalar.activation(
                            r1lg[oc][:, 36 + a:36 + b], ps[:, 0:b - a],
                            AF.Relu, bias=bias_sb["lg1"][:, oc:oc + 1],
                            scale=1.0)

                    emit_convgroup([(w_lg1, rcur + rlr, 8)], R_RLG, cps,
                                   r1lg_evac)
                    for c in range(4):
                        zero_pads_post(nc, r1lg[c], R_RLG)

                    def lf_evac(oc, a, b, ps):
                        nc.scalar.activation(
                            lf[oc][:, 36 + a:36 + b], ps[:, 0:b - a],
                            AF.Relu, bias=bias_sb["lgf"][:, oc:oc + 1],
                            scale=1.0)

                    emit_convgroup([(w_lgd, cur + lrp, 8), (w_lg2, r1lg, 4)],
                                   R_LF, cps, lf_evac)
                    for c in range(4):
                        zero_pads_post(nc, lf[c], R_LF)

                    # ig fuse
                    rsf = [cvp.tile([128, PL(R_SF)], F32R, tag="rsf", name="rsf")
                           for _ in range(4)]
                    rlf = [cvp.tile([128, PL(R_SF)], F32R, tag="rlf", name="rlf")
                           for _ in range(4)]
                    for c in range(4):
                        nc.scalar.activation(rsf[c][:], sf[c][:], AF.Relu,
                                             bias=0.0, scale=1.0)
                        nc.scalar.activation(rlf[c][:], lf[c][:], AF.Relu,
                                             bias=0.0, scale=1.0)
                    r1ig = [cvp.tile([128, PL(R_RIG)], F32R, tag="r1ig", name="r1ig")
                            for _ in range(4)]
                    fus_t = [fp.tile([128, 512], F32, tag="fus", name="fus")
                             for _ in range(4)]
                    for c in range(4):
                        fus[c] = fus_t[c]

                    def r1ig_evac(oc, a, b, ps):
                        nc.scalar.activation(
                            r1ig[oc][:, 36 + a:36 + b], ps[:, 0:b - a],
                            AF.Relu, bias=bias_sb["ig1"][:, oc:oc + 1],
                            scale=1.0)

                    emit_convgroup([(w_ig1, rcur + rsf + rlf, 12)], R_RIG,
                                   cps, r1ig_evac)
                    for c in range(4):
                        zero_pads_post(nc, r1ig[c], R_RIG)

                    def fus_evac(oc, a, b, ps):
                        r0, nr = a // 34, (b - a) // 34
                        nc.scalar.activation(
                            fus_t[oc][:, r0 * 32:(r0 + nr) * 32].rearrange(
                                "p (r c) -> p r c", c=32),
                            ps[:, 0:b - a].rearrange(
                                "p (r c) -> p r c", c=34)[:, :, 1:33],
                            AF.Relu, bias=bias_sb["igf"][:, oc:oc + 1],
                            scale=1.0)

                    emit_convgroup([(w_igd, cur + sf + lf, 12),
                                    (w_ig2, r1ig, 4)], R_OUT, cps, fus_evac)

                    # fusion -> cc_in (contiguous)
                    for c in range(4):
                        nc.sync.dma_start(
                            out=cc_in[idx][c * 128:(c + 1) * 128, :],
                            in_=fus_t[c][:])

                # ---- exchange + frame update ----
                nc.gpsimd.collective_compute(
                    "AllGather", ALU.bypass,
                    ins=[cc_in[idx][:]], outs=[cc_out[idx][:]],
                    replica_groups=PAIRS)
                # scheduler fence here (not at frame start): the next frame's
                # attention prefix may overlap this frame's post-gather tail,
                # while allocations can't hoist past the prior frame's compute
                # (prevents pool-slot deadlocks)
                tc.no_sync_barrier()

                with (
                    tc.tile_pool(name="post", bufs=4) as pop,
                    tc.tile_pool(name="ftps", bufs=2, space="PSUM") as ftps,
                    tc.tile_pool(name="aqps", bufs=1, space="PSUM") as aqps,
                ):
                    newf = [pop.tile([128, HW], F32, tag="nf", name="nf")
                            for _ in range(4)]
                    for c in range(4):
                        # own half (already canonical local rows)
                        nc.vector.tensor_copy(newf[c][:, 0:512], fus[c][:])
                        # sibling half: contiguous loads, row-reverse via DVE
                        g0 = pop.tile([128, 512], F32, tag="g0", name="g0")
                        g1 = pop.tile([128, 512], F32, tag="g1", name="g1")
                        for s, gt in ((0, g0), (1, g1)):
                            nc.sync.dma_start(
                                out=gt[:],
                                in_=cc_out[idx][s, c * 128:(c + 1) * 128, :])

                        def rev(gt):
                            v = gt[:].rearrange("p (r c) -> p r c", c=32)
                            return bass.AP(
                                tensor=v.tensor, offset=v.offset + 15 * 32,
                                ap=[v.ap[0], [-32, 16], [1, 32]])

                        out_v = newf[c][:, 512:HW].rearrange(
                            "p (r c) -> p r c", c=32)
                        nc.vector.tensor_scalar(
                            out=out_v, in0=rev(g0),
                            scalar1=w0_sb[:], scalar2=None, op0=ALU.mult)
                        nc.vector.scalar_tensor_tensor(
                            out=out_v, in0=rev(g1),
                            scalar=w1_sb[:], op0=ALU.mult, op1=ALU.add,
                            in1=out_v)
                        nc.sync.dma_start(
                            out=fw[idx][c * 128:(c + 1) * 128, :],
                            in_=newf[c][:])

                    # featT update
                    for pb in range(8):
                        ft = pop.tile([128, 512], F32, tag="ft", name="ft")
                        for c in range(4):
                            pst = ftps.tile([128, 128], F32, tag="ftp", name="ftp")
                            nc.tensor.transpose(
                                pst[:], newf[c][:, pb * 128:(pb + 1) * 128],
                                ident[:])
                            nc.vector.tensor_copy(
                                ft[:, c * 128:(c + 1) * 128], pst[:])
                        nc.sync.dma_start(
                            out=fTw[idx][pb * 128:(pb + 1) * 128, :],
                            in_=ft[:])

                    # asq update: -(sum_c f^2)/sqrt(C)
                    sq = [pop.tile([128, HW], F32R, tag="sq", name="sq")
                          for _ in range(4)]
                    for c in range(4):
                        nc.vector.tensor_tensor(out=sq[c][:], in0=newf[c][:],
                                                in1=newf[c][:], op=ALU.mult)
                    arow = pop.tile([1, HW], F32, tag="arow", name="arow")
                    for hhalf in range(2):
                        ps2 = aqps.tile([2, 512], F32, tag="aq", name="aq")
                        for c in range(4):
                            nc.tensor.matmul(
                                ps2[:], ones2[:],
                                sq[c][:, hhalf * 512:(hhalf + 1) * 512],
                                start=(c == 0), stop=(c == 3))
                        nc.scalar.activation(
                            arow[0:1, hhalf * 512:(hhalf + 1) * 512],
                            ps2[0:1, :], AF.Copy, bias=0.0, scale=-1.0 / SQC)
                    nc.sync.dma_start(out=asqw[idx][:], in_=arow[0:1, :])

            # frames never updated (debug builds with frames < T): copy input
            for t in range(frames, T):
                nc.sync.dma_start(out=fw[t][:], in_=feat0[t])

    nc.finalize()
    return nc


def zero_pads_post(nc, t, r):
    """Zero the pad columns of a conv-output tile after its evacs wrote the
    full [0, 34r) span (pre/postamble were zeroed at allocation... here)."""
    f = t[:].bitcast(F32)
    nc.vector.memset(f[:, 0:36], 0.0)
    nc.vector.memset(f[:, 36 + 34 * r:], 0.0)
    v3 = f[:, 36:36 + 34 * r].rearrange("p (r c) -> p r c", c=34)
    nc.vector.memset(v3[:, :, 0:1], 0.0)
    nc.vector.memset(v3[:, :, 33:34], 0.0)


# ---------------- host side ----------------

def _fold_bn(p, pre):
    eps = 1e-5
    w1 = p[pre + "_conv1_w"]
    b1c = p[pre + "_conv1_b"]
    w2 = p[pre + "_conv2_w"]
    b2c = p[pre + "_conv2_b"]
    wd = p[pre + "_down_w"]
    bd = p[pre + "_down_b"]
    g1, bb1, m1, v1 = (p[pre + "_bn1_" + s] for s in "gbmv")
    g2, bb2, m2, v2 = (p[pre + "_bn2_" + s] for s in "gbmv")
    s1 = g1 / np.sqrt(v1 + eps)
    s2 = g2 / np.sqrt(v2 + eps)
    w1f = w1 * s1[:, None, None, None]
    b1f = b1c * s1 + bb1 - m1 * s1
    w2f = w2 * s2[:, None, None, None]
    b2f = b2c * s2 + bb2 - m2 * s2
    bfin = bd + b2f

    def to9(w):
        return np.ascontiguousarray(
            w.transpose(2, 3, 1, 0).reshape(9, w.shape[1], w.shape[0])
        ).astype(np.float32)

    return (to9(w1f), b1f.astype(np.float32), to9(w2f), to9(wd),
            bfin.astype(np.float32))


def _flip9(w9):
    return np.ascontiguousarray(
        w9.reshape(3, 3, *w9.shape[1:])[::-1].reshape(w9.shape))


_NC = None


def _get_nc():
    global _NC
    if _NC is None:
        _NC = build()
    return _NC


def make_in_maps(inputs):
    feats = np.asarray(inputs["features"], dtype=np.float32)
    p = {k: np.asarray(v, dtype=np.float32) for k, v in inputs.items()
         if k != "features"}
    lg = _fold_bn(p, "lg")
    ig = _fold_bn(p, "ig")
    lg_f = tuple(_flip9(w) if w.ndim == 3 else w for w in lg)
    ig_f = tuple(_flip9(w) if w.ndim == 3 else w for w in ig)

    in_maps = []
    for core in range(N_CORES):
        b, h = core // 2, core % 2
        fb = feats[b] if h == 0 else feats[b][:, :, ::-1, :]
        f0 = np.ascontiguousarray(fb.reshape(T, C, HW))
        fT0 = np.ascontiguousarray(f0.transpose(0, 2, 1))
        a0 = -(f0.astype(np.float64) ** 2).sum(1) / SQC
        wl = lg if h == 0 else lg_f
        wi = ig if h == 0 else ig_f
        in_maps.append({
            "feat0": f0,
            "featT0": fT0,
            "asq0": a0.astype(np.float32),
            "w_lg1": wl[0], "b_lg1": wl[1], "w_lg2": wl[2],
            "w_lgd": wl[3], "b_lgf": wl[4],
            "w_ig1": wi[0], "b_ig1": wi[1], "w_ig2": wi[2],
            "w_igd": wi[3], "b_igf": wi[4],
            "selw": np.array([0.0, 1.0] if h == 0 else [1.0, 0.0],
                             np.float32),
        })
    return in_maps


def kernel(**inputs):
    nc = _get_nc()
    in_maps = make_in_maps(inputs)
    res = run_bass_kernel_spmd(nc, in_maps, list(range(N_CORES)))
    out = np.zeros((B, T, C, H, W), np.float32)
    for b in range(B):
        for t in range(T):
            out[b, t] = res.results[2 * b][f"feat_out{t}"].reshape(C, H, W)
    return out

